# revision 1
# baseline (speedup 1.0000x reference)
"""FFM layer (linear + field-aware FM interaction) on 8 Trainium2 cores.

Sharding: row-parallel GEMM over the feature axis. Core c holds a
13056-feature stripe of inputs^T ([13056, 1024]) and of the combined
weight matrix G = [v.reshape(F, 312) | w] ([13056, 313]). Each core
computes its partial inputs_c^T.T @ G_c -> [1024, 313] with fp32
(float32r PE mode) matmuls accumulated in PSUM over 102 k-tiles.
The host sums the 8 partials and applies the cheap FM epilogue
(sum-square identity) in fp64, returning [1024, 1] fp32.
"""

import numpy as np

B = 1024
F = 104013
FIELD = 39
K = 8
NV = FIELD * K          # 312 interaction columns
NL = NV                 # linear column index
NK = NV + 2             # + linear column + 1 zero pad col (fp32r needs even N)
N_CORES = 8
KT = 102                # 128-row k-tiles per core
FPC = KT * 128          # 13056 padded features per core
CH = 3                  # k-tiles per DMA chunk
BUFS = 6                # SBUF double-buffer depth for streamed chunks
DMA_ENGINE = "sync"     # "sync" (HWDGE) or "gpsimd" (SWDGE)
WARMUP_LDW = 0          # dummy ldweights before the stream (PE pre-warm)
FILLER_LDW = 0          # dummy ldweights per chunk (keep HAM warm in stalls)
G_DMA = "sync"          # engine for g-stream DMAs
OUT_DMA = "sync"        # engine for output DMAs
POOL_MODE = "queue"     # TileContext pool_alloc_mode (ring SBUF alloc: fewer overlap-dep stalls)

_nc = None
last_exec_time_ns = None


def _build():
    from concourse import bass, mybir, tile, bacc

    nc = bacc.Bacc("TRN2", num_devices=N_CORES)
    f32 = mybir.dt.float32
    f32r = mybir.dt.float32r

    xt = nc.dram_tensor("xt", [FPC, B], f32r, kind="ExternalInput")
    g = nc.dram_tensor("g", [FPC, NK], f32r, kind="ExternalInput")
    out = nc.dram_tensor("out", [B, NK], f32, kind="ExternalOutput")

    xt_r = xt.rearrange("(t p) m -> p t m", p=128)  # [128, KT, B]
    g_r = g.rearrange("(t p) n -> p t n", p=128)    # [128, KT, NK]

    with tile.TileContext(nc, pool_alloc_mode=POOL_MODE) as tc:
        with (
            tc.tile_pool(name="xt", bufs=BUFS) as xt_pool,
            tc.tile_pool(name="g", bufs=BUFS) as g_pool,
            tc.tile_pool(name="acc", bufs=1, space=bass.MemorySpace.PSUM) as psum_pool,
            tc.tile_pool(name="o", bufs=1) as out_pool,
        ):
            n_b = B // 128
            accs = [
                psum_pool.tile([128, NK], f32, tag=f"acc{b}", name=f"acc{b}")
                for b in range(n_b)
            ]
            # Scratch bf16 weight tile: dummy ldweights on it keep the PE
            # HAM activity monitor warm during DMA stalls. The loaded
            # weights are never used (every real fp32r matmul self-loads).
            if WARMUP_LDW or FILLER_LDW:
                bf16 = mybir.dt.bfloat16
                warm = out_pool.tile([128, 128], bf16, tag="warm", name="warm")
                nc.gpsimd.memset(warm[:], 0.0)
                for _ in range(WARMUP_LDW):
                    nc.tensor.ldweights(warm[:])
            dma = nc.sync if DMA_ENGINE == "sync" else nc.gpsimd
            dma_g = nc.sync if G_DMA == "sync" else nc.gpsimd
            dma_out = nc.sync if OUT_DMA == "sync" else nc.gpsimd
            # Graduated chunks: tiny first chunks so the PE starts as soon
            # as possible, steady CH-tile chunks afterwards.
            chunks = []
            for n in [1, 1, 2, 2]:
                if sum(chunks) + n <= KT:
                    chunks.append(n)
            while KT - sum(chunks) > 0:
                chunks.append(min(CH, KT - sum(chunks)))
            kc = 0
            for ci, n in enumerate(chunks):
                last_chunk = ci == len(chunks) - 1
                xt_t = xt_pool.tile([128, n, B], f32r, tag="xt", name=f"xt{kc}")
                dma.dma_start(xt_t[:], xt_r[:, kc : kc + n, :])
                g_t = g_pool.tile([128, n, NK], f32r, tag="g", name=f"gt{kc}")
                dma_g.dma_start(g_t[:], g_r[:, kc : kc + n, :])
                # b-major in the last chunk so each acc finishes (and its
                # copy-out can start) as early as possible.
                order = (
                    [(i, b) for b in range(n_b) for i in range(n)]
                    if last_chunk
                    else [(i, b) for i in range(n) for b in range(n_b)]
                )
                for i, b in order:
                    k = kc + i
                    nc.tensor.matmul(
                        accs[b][:],
                        xt_t[:, i, b * 128 : (b + 1) * 128],
                        g_t[:, i, :],
                        start=(k == 0),
                        stop=(k == KT - 1),
                    )
                if FILLER_LDW and not last_chunk:
                    for _ in range(FILLER_LDW):
                        nc.tensor.ldweights(warm[:])
                kc += n
            for b in range(n_b):
                o = out_pool.tile([128, NK], f32, tag=f"o{b}", name=f"ot{b}")
                nc.vector.tensor_copy(o[:], accs[b][:])
                dma_out.dma_start(out[b * 128 : (b + 1) * 128, :], o[:])
    nc.compile()
    return nc


def _get_nc():
    global _nc
    if _nc is None:
        _nc = _build()
    return _nc


def kernel(inputs, w0, w, v, _trace=False):
    global last_exec_time_ns
    from concourse.bass_utils import run_bass_kernel_spmd

    inputs = np.asarray(inputs, dtype=np.float32)
    w0 = np.asarray(w0, dtype=np.float32)
    w = np.asarray(w, dtype=np.float32)
    v = np.asarray(v, dtype=np.float32)

    # G = [v | w] : [F, 313], zero-padded to 8 * 13056 rows
    G = np.zeros((N_CORES * FPC, NK), dtype=np.float32)
    G[:F, :NV] = v.reshape(F, NV)
    G[:F, NL] = w[:, 0]
    # inputs^T, zero-padded the same way
    XT = np.zeros((N_CORES * FPC, B), dtype=np.float32)
    XT[:F] = inputs.T

    in_maps = [
        {"xt": XT[c * FPC : (c + 1) * FPC], "g": G[c * FPC : (c + 1) * FPC]}
        for c in range(N_CORES)
    ]
    nc = _get_nc()
    import os

    prev = os.environ.get("BASS_NEVER_TRACE")
    if not _trace:
        # Profiling needs an NTFF hook this container may not have; make
        # sure a stray BASS_TRACE env var can't pull us down that path.
        os.environ["BASS_NEVER_TRACE"] = "1"
    try:
        import time

        res = None
        for attempt in range(3):
            try:
                res = run_bass_kernel_spmd(
                    nc, in_maps, list(range(N_CORES)), trace=_trace
                )
                break
            except Exception:
                # Transient device wedges (NRT_EXEC_UNIT_UNRECOVERABLE) have
                # been observed on this shared box; retry before giving up.
                if attempt == 2:
                    raise
                time.sleep(10)
    finally:
        if not _trace:
            if prev is None:
                os.environ.pop("BASS_NEVER_TRACE", None)
            else:
                os.environ["BASS_NEVER_TRACE"] = prev
    last_exec_time_ns = res.exec_time_ns

    total = np.zeros((B, NK), dtype=np.float64)
    for c in range(N_CORES):
        total += res.results[c]["out"]

    field_f = total[:, :NV].reshape(B, FIELD, K)
    linear = total[:, NL] + np.float64(w0[0])
    s = field_f.sum(axis=1)                                     # [B, K]
    inter = 0.5 * ((s * s).sum(axis=-1) - (field_f * field_f).sum(axis=(1, 2)))
    return (linear + inter)[:, None].astype(np.float32)



# revision 2
# speedup vs baseline: 1.4899x; 1.4899x over previous
"""FFM layer (linear + field-aware FM interaction) on 8 Trainium2 cores.

Sharding: row-parallel GEMM over the feature axis. Core c holds a
13056-feature stripe of inputs^T ([13056, 1024]) and of the combined
weight matrix G = [v.reshape(F, 312) | w] ([13056, 314]), both cast to
bf16 on host (rel err ~0.2%, tolerance 2e-2). Each core computes its
partial inputs_c^T.T @ G_c -> [1024, 314] with bf16 matmuls accumulated
in fp32 PSUM over 102 k-tiles. The host sums the 8 partials and applies
the cheap FM epilogue (sum-square identity) in fp64, returning
[1024, 1] fp32.

bf16 halves HBM->SBUF traffic vs the fp32 baseline (which the trace
showed was 90% DMA-active / DMA-bound at ~310 GB/s/core).
"""

import numpy as np

B = 1024
F = 104013
FIELD = 39
K = 8
NV = FIELD * K          # 312 interaction columns
NL = NV                 # linear column index
NK = NV + 2             # + linear column + 1 zero pad col
N_CORES = 8
KT = 102                # 128-row k-tiles per core
FPC = KT * 128          # 13056 padded features per core
CH = 3                  # k-tiles per DMA chunk
BUFS = 6                # SBUF double-buffer depth for streamed chunks
DMA_ENGINE = "sync"     # "sync" (HWDGE) or "gpsimd" (SWDGE)
G_DMA = "sync"          # engine for g-stream DMAs
OUT_DMA = "sync"        # engine for output DMAs
POOL_MODE = "queue"     # TileContext pool_alloc_mode (ring SBUF alloc)

_nc = None
last_exec_time_ns = None


def _build():
    from concourse import bass, mybir, tile, bacc

    nc = bacc.Bacc("TRN2", num_devices=N_CORES)
    f32 = mybir.dt.float32
    bf16 = mybir.dt.bfloat16

    xt = nc.dram_tensor("xt", [FPC, B], bf16, kind="ExternalInput")
    g = nc.dram_tensor("g", [FPC, NK], bf16, kind="ExternalInput")
    out = nc.dram_tensor("out", [B, NK], f32, kind="ExternalOutput")

    xt_r = xt.rearrange("(t p) m -> p t m", p=128)  # [128, KT, B]
    g_r = g.rearrange("(t p) n -> p t n", p=128)    # [128, KT, NK]

    with tile.TileContext(nc, pool_alloc_mode=POOL_MODE) as tc:
        with (
            tc.tile_pool(name="xt", bufs=BUFS) as xt_pool,
            tc.tile_pool(name="g", bufs=BUFS) as g_pool,
            tc.tile_pool(name="acc", bufs=1, space=bass.MemorySpace.PSUM) as psum_pool,
            tc.tile_pool(name="o", bufs=1) as out_pool,
        ):
            n_b = B // 128
            accs = [
                psum_pool.tile([128, NK], f32, tag=f"acc{b}", name=f"acc{b}")
                for b in range(n_b)
            ]
            dma = nc.sync if DMA_ENGINE == "sync" else nc.gpsimd
            dma_g = nc.sync if G_DMA == "sync" else nc.gpsimd
            dma_out = nc.sync if OUT_DMA == "sync" else nc.gpsimd
            # Graduated chunks: tiny first chunks so the PE starts as soon
            # as possible, steady CH-tile chunks afterwards.
            chunks = []
            for n in [1, 1, 2, 2]:
                if sum(chunks) + n <= KT:
                    chunks.append(n)
            while KT - sum(chunks) > 0:
                chunks.append(min(CH, KT - sum(chunks)))
            kc = 0
            for ci, n in enumerate(chunks):
                last_chunk = ci == len(chunks) - 1
                xt_t = xt_pool.tile([128, n, B], bf16, tag="xt", name=f"xt{kc}")
                dma.dma_start(xt_t[:], xt_r[:, kc : kc + n, :])
                g_t = g_pool.tile([128, n, NK], bf16, tag="g", name=f"gt{kc}")
                dma_g.dma_start(g_t[:], g_r[:, kc : kc + n, :])
                # b-major in the last chunk so each acc finishes (and its
                # copy-out can start) as early as possible.
                order = (
                    [(i, b) for b in range(n_b) for i in range(n)]
                    if last_chunk
                    else [(i, b) for i in range(n) for b in range(n_b)]
                )
                for i, b in order:
                    k = kc + i
                    nc.tensor.matmul(
                        accs[b][:],
                        xt_t[:, i, b * 128 : (b + 1) * 128],
                        g_t[:, i, :],
                        start=(k == 0),
                        stop=(k == KT - 1),
                    )
                kc += n
            for b in range(n_b):
                o = out_pool.tile([128, NK], f32, tag=f"o{b}", name=f"ot{b}")
                nc.vector.tensor_copy(o[:], accs[b][:])
                dma_out.dma_start(out[b * 128 : (b + 1) * 128, :], o[:])
    nc.compile()
    return nc


def _get_nc():
    global _nc
    if _nc is None:
        _nc = _build()
    return _nc


def kernel(inputs, w0, w, v, _trace=False):
    global last_exec_time_ns
    import ml_dtypes
    from concourse.bass_utils import run_bass_kernel_spmd

    bf16 = ml_dtypes.bfloat16
    inputs = np.asarray(inputs, dtype=np.float32)
    w0 = np.asarray(w0, dtype=np.float32)
    w = np.asarray(w, dtype=np.float32)
    v = np.asarray(v, dtype=np.float32)

    # G = [v | w] : [F, 314], zero-padded to 8 * 13056 rows, bf16
    G = np.zeros((N_CORES * FPC, NK), dtype=bf16)
    G[:F, :NV] = v.reshape(F, NV).astype(bf16)
    G[:F, NL] = w[:, 0].astype(bf16)
    # inputs^T, zero-padded the same way, bf16
    XT = np.zeros((N_CORES * FPC, B), dtype=bf16)
    XT[:F] = inputs.T.astype(bf16)

    in_maps = [
        {"xt": XT[c * FPC : (c + 1) * FPC], "g": G[c * FPC : (c + 1) * FPC]}
        for c in range(N_CORES)
    ]
    nc = _get_nc()
    import os

    prev = os.environ.get("BASS_NEVER_TRACE")
    if not _trace:
        # Profiling needs an NTFF hook this container may not have; make
        # sure a stray BASS_TRACE env var can't pull us down that path.
        os.environ["BASS_NEVER_TRACE"] = "1"
    try:
        import time

        res = None
        for attempt in range(3):
            try:
                res = run_bass_kernel_spmd(
                    nc, in_maps, list(range(N_CORES)), trace=_trace
                )
                break
            except Exception:
                # Transient device wedges (NRT_EXEC_UNIT_UNRECOVERABLE) have
                # been observed on this shared box; retry before giving up.
                if attempt == 2:
                    raise
                time.sleep(10)
    finally:
        if not _trace:
            if prev is None:
                os.environ.pop("BASS_NEVER_TRACE", None)
            else:
                os.environ["BASS_NEVER_TRACE"] = prev
    last_exec_time_ns = res.exec_time_ns

    total = np.zeros((B, NK), dtype=np.float64)
    for c in range(N_CORES):
        total += res.results[c]["out"]

    field_f = total[:, :NV].reshape(B, FIELD, K)
    linear = total[:, NL] + np.float64(w0[0])
    s = field_f.sum(axis=1)                                     # [B, K]
    inter = 0.5 * ((s * s).sum(axis=-1) - (field_f * field_f).sum(axis=(1, 2)))
    return (linear + inter)[:, None].astype(np.float32)


# revision 4
# speedup vs baseline: 1.7416x; 1.1690x over previous
"""FFM layer (linear + field-aware FM interaction) on 8 Trainium2 cores.

Sharding: row-parallel GEMM over the feature axis. Core c holds a
13056-feature stripe of inputs^T and of the combined weight matrix
G = [v.reshape(F, 312) | w], both bf16 (rel err ~0.3%, tolerance 2e-2).
Each core computes its partial inputs_c^T.T @ G_c -> [1024, 314] with
bf16 matmuls accumulated in fp32 PSUM over 102 k-tiles. The host sums
the 8 partials and applies the cheap FM epilogue (sum-square identity)
in fp64, returning [1024, 1] fp32.

Layout: one interleaved DRAM stream per core, [128, KT*1344] bf16.
Each k-tile slot holds [g(314) | pad(6) | xt(1024)] per partition, so
a chunk is one dma_start with large contiguous descriptors (n*2688 B
per partition) and 16B-aligned lhsT/rhs slices.

Timeline per trace analysis: PE-bound at ~160 ns per (LDWEIGHTS,
MATMUL) pair (314-cycle stream @2.4GHz + ~29 ns weight-swap bubble).
Warm-up LDWEIGHTS keep the PE HAM activity monitor busy during the
initial DMA wait so real matmuls start at 2.4 GHz. PSUM->SBUF copies
alternate vector/scalar engines; output goes out in 2 grouped DMAs.
"""

import numpy as np

B = 1024
F = 104013
FIELD = 39
K = 8
NV = FIELD * K          # 312 interaction columns
NL = NV                 # linear column index
NK = NV + 2             # + linear column + 1 zero pad col
N_CORES = 8
KT = 102                # 128-row k-tiles per core
FPC = KT * 128          # 13056 padded features per core
GOFF = 320              # xt offset (elems) within a k-tile slot
SLOT = GOFF + B         # 1344 elems per k-tile slot (2688 B, 16B-aligned)
CH = 3                  # k-tiles per DMA chunk
BUFS = 6                # SBUF double-buffer depth for streamed chunks
WARMUP_LDW = 28         # dummy ldweights before the stream (HAM pre-warm)

_nc = None
last_exec_time_ns = None


def _build():
    from concourse import bass, mybir, tile, bacc

    nc = bacc.Bacc("TRN2", num_devices=N_CORES)
    f32 = mybir.dt.float32
    bf16 = mybir.dt.bfloat16

    xg = nc.dram_tensor("xg", [128, KT * SLOT], bf16, kind="ExternalInput")
    out = nc.dram_tensor("out", [B, NK], f32, kind="ExternalOutput")
    out_r = out.rearrange("(j p) n -> p j n", p=128)  # [128, 8, NK]

    with tile.TileContext(nc, pool_alloc_mode="queue") as tc:
        with (
            tc.tile_pool(name="xg", bufs=BUFS) as xg_pool,
            tc.tile_pool(name="acc", bufs=1, space=bass.MemorySpace.PSUM) as psum_pool,
            tc.tile_pool(name="o", bufs=1) as out_pool,
        ):
            n_b = B // 128
            accs = [
                psum_pool.tile([128, NK], f32, tag=f"acc{b}", name=f"acc{b}")
                for b in range(n_b)
            ]
            # Dummy ldweights on a zeroed tile keep the PE HAM activity
            # monitor warm while the first chunks stream in, so the real
            # matmuls start at 2.4 GHz instead of ramping from 1.2.
            if WARMUP_LDW:
                warm = out_pool.tile([128, 128], bf16, tag="warm", name="warm")
                nc.gpsimd.memset(warm[:], 0.0)
                for _ in range(WARMUP_LDW):
                    nc.tensor.ldweights(warm[:])
            # Graduated chunks: tiny first chunks so the PE starts as soon
            # as possible, steady CH-tile chunks afterwards.
            chunks = []
            for n in [1, 1, 2, 2]:
                if sum(chunks) + n <= KT:
                    chunks.append(n)
            while KT - sum(chunks) > 0:
                chunks.append(min(CH, KT - sum(chunks)))
            kc = 0
            for ci, n in enumerate(chunks):
                last_chunk = ci == len(chunks) - 1
                t = xg_pool.tile([128, n * SLOT], bf16, tag="xg", name=f"xg{kc}")
                nc.sync.dma_start(t[:], xg[:, kc * SLOT : (kc + n) * SLOT])
                # b-major in the last chunk so each acc finishes (and its
                # copy-out can start) as early as possible.
                order = (
                    [(i, b) for b in range(n_b) for i in range(n)]
                    if last_chunk
                    else [(i, b) for i in range(n) for b in range(n_b)]
                )
                for i, b in order:
                    k = kc + i
                    nc.tensor.matmul(
                        accs[b][:],
                        t[:, i * SLOT + GOFF + b * 128 : i * SLOT + GOFF + (b + 1) * 128],
                        t[:, i * SLOT : i * SLOT + NK],
                        start=(k == 0),
                        stop=(k == KT - 1),
                    )
                kc += n
            # PSUM -> SBUF copies alternate vector/scalar so they drain
            # ~2x faster; outputs leave in 2 grouped DMAs (fewer serialized
            # HWDGE triggers on the sync queue).
            o = out_pool.tile([128, n_b * NK], f32, tag="o", name="o")
            for b in range(n_b):
                if b % 2 == 0:
                    nc.vector.tensor_copy(o[:, b * NK : (b + 1) * NK], accs[b][:])
                else:
                    nc.scalar.copy(o[:, b * NK : (b + 1) * NK], accs[b][:])
                if b == 3:
                    nc.sync.dma_start(out_r[:, 0:4, :], o[:, : 4 * NK])
            nc.sync.dma_start(out_r[:, 4:8, :], o[:, 4 * NK :])
    nc.compile()
    return nc


def _get_nc():
    global _nc
    if _nc is None:
        _nc = _build()
    return _nc


def _pack_inputs(inputs, w, v):
    """Build per-core interleaved [128, KT*SLOT] bf16 streams."""
    import ml_dtypes

    bf16 = ml_dtypes.bfloat16
    FP = N_CORES * FPC
    XG = np.zeros((N_CORES, 128, KT, SLOT), dtype=bf16)
    # g part: [v | w] -> rows are features, cols are [312 v-cols, w, pad]
    Gv = XG[..., :NK].reshape(N_CORES, 128, KT, NK)
    G = np.zeros((FP, NK), dtype=bf16)
    G[:F, :NV] = v.reshape(F, NV).astype(bf16)
    G[:F, NL] = w[:, 0].astype(bf16)
    Gv[:] = G.reshape(N_CORES, KT, 128, NK).transpose(0, 2, 1, 3)
    # xt part: inputs^T
    XT = np.zeros((FP, B), dtype=bf16)
    XT[:F] = inputs.T.astype(bf16)
    XG[..., GOFF:] = XT.reshape(N_CORES, KT, 128, B).transpose(0, 2, 1, 3)
    return XG.reshape(N_CORES, 128, KT * SLOT)


def kernel(inputs, w0, w, v, _trace=False):
    global last_exec_time_ns
    from concourse.bass_utils import run_bass_kernel_spmd

    inputs = np.asarray(inputs, dtype=np.float32)
    w0 = np.asarray(w0, dtype=np.float32)
    w = np.asarray(w, dtype=np.float32)
    v = np.asarray(v, dtype=np.float32)

    XG = _pack_inputs(inputs, w, v)
    in_maps = [{"xg": XG[c]} for c in range(N_CORES)]
    nc = _get_nc()
    import os

    prev = os.environ.get("BASS_NEVER_TRACE")
    if not _trace:
        # Profiling needs an NTFF hook this container may not have; make
        # sure a stray BASS_TRACE env var can't pull us down that path.
        os.environ["BASS_NEVER_TRACE"] = "1"
    try:
        import time

        res = None
        for attempt in range(3):
            try:
                res = run_bass_kernel_spmd(
                    nc, in_maps, list(range(N_CORES)), trace=_trace
                )
                break
            except Exception:
                # Transient device wedges have been observed on shared
                # boxes; retry before giving up.
                if attempt == 2:
                    raise
                time.sleep(10)
    finally:
        if not _trace:
            if prev is None:
                os.environ.pop("BASS_NEVER_TRACE", None)
            else:
                os.environ["BASS_NEVER_TRACE"] = prev
    last_exec_time_ns = res.exec_time_ns

    total = np.zeros((B, NK), dtype=np.float64)
    for c in range(N_CORES):
        total += res.results[c]["out"]

    field_f = total[:, :NV].reshape(B, FIELD, K)
    linear = total[:, NL] + np.float64(w0[0])
    s = field_f.sum(axis=1)                                     # [B, K]
    inter = 0.5 * ((s * s).sum(axis=-1) - (field_f * field_f).sum(axis=(1, 2)))
    return (linear + inter)[:, None].astype(np.float32)


# revision 9
# speedup vs baseline: 1.7720x; 1.0174x over previous
"""FFM layer (linear + field-aware FM interaction) on 8 Trainium2 cores.

Sharding: row-parallel GEMM over the feature axis. Core c holds a
13056-feature stripe of inputs^T and of the combined weight matrix
G = [v.reshape(F, 312) | w], both bf16 (rel err ~0.3%, tolerance 2e-2).
Each core computes its partial inputs_c^T.T @ G_c -> [1024, 314] with
bf16 matmuls accumulated in fp32 PSUM over 102 k-tiles. The host sums
the 8 partials and applies the cheap FM epilogue (sum-square identity)
in fp64, returning [1024, 1] fp32.

Layout: one interleaved DRAM stream per core, [128, KT*1344] bf16.
Each k-tile slot holds [g(314) | pad(6) | xt(1024)] per partition, so
a chunk is one dma_start with large contiguous descriptors (n*2688 B
per partition) and 16B-aligned lhsT/rhs slices.

Timeline per trace analysis: PE-bound at ~160 ns per (LDWEIGHTS,
MATMUL) pair (314-cycle stream @2.4GHz + ~29 ns weight-swap bubble).
Warm-up LDWEIGHTS keep the PE HAM activity monitor busy during the
initial DMA wait so real matmuls start at 2.4 GHz. PSUM->SBUF copies
alternate vector/scalar engines; output goes out in 2 grouped DMAs.
"""

import numpy as np

B = 1024
F = 104013
FIELD = 39
K = 8
NV = FIELD * K          # 312 interaction columns
NL = NV                 # linear column index
NK = NV + 2             # + linear column + 1 zero pad col
N_CORES = 8
KT = 102                # 128-row k-tiles per core
FPC = KT * 128          # 13056 padded features per core
GOFF = 320              # xt offset (elems) within a k-tile slot
SLOT = GOFF + B         # 1344 elems per k-tile slot (2688 B, 16B-aligned)
CH = 3                  # k-tiles per DMA chunk
BUFS = 6                # SBUF double-buffer depth for streamed chunks
WARM_MM = 22            # dummy matmuls before the stream (HAM pre-warm)

_nc = None
last_exec_time_ns = None


def _build():
    from concourse import bass, mybir, tile, bacc

    nc = bacc.Bacc("TRN2", num_devices=N_CORES)
    f32 = mybir.dt.float32
    bf16 = mybir.dt.bfloat16

    xg = nc.dram_tensor("xg", [128, KT * SLOT], bf16, kind="ExternalInput")
    # Output stays partition-major ([128, 8*NK]: partition p, then batch
    # tile j, then column n) so the two output DMAs have 10KB/5KB
    # contiguous per-partition runs; the host untransposes.
    out = nc.dram_tensor("out", [128, (B // 128) * NK], f32, kind="ExternalOutput")

    with tile.TileContext(nc, pool_alloc_mode="queue") as tc:
        with (
            tc.tile_pool(name="xg", bufs=BUFS) as xg_pool,
            tc.tile_pool(name="acc", bufs=1, space=bass.MemorySpace.PSUM) as psum_pool,
            tc.tile_pool(name="o", bufs=1) as out_pool,
        ):
            n_b = B // 128
            accs = [
                psum_pool.tile([128, NK], f32, tag=f"acc{b}", name=f"acc{b}")
                for b in range(n_b)
            ]
            # Dummy matmuls on a zeroed tile keep the PE busy (HAM
            # activity monitor warm) while the first chunks stream in, so
            # the real matmuls run at 2.4 GHz from the start instead of
            # spending their first ~3.4us at 1.2. They write acc0 as a
            # self-contained start/stop group; the real k=0 matmul
            # (start=True) resets it.
            if WARM_MM:
                warm = out_pool.tile([128, 320], bf16, tag="warm", name="warm")
                nc.gpsimd.memset(warm[:], 0.0)
                for _ in range(WARM_MM):
                    nc.tensor.matmul(
                        accs[0][:, :NK],
                        warm[:, :128],
                        warm[:, :NK],
                        start=True,
                        stop=True,
                    )
            # Graduated chunks: tiny first chunks so the PE starts as soon
            # as possible, steady CH-tile chunks afterwards, and a small
            # final chunk so the accs finish staggered (copy-out overlap).
            chunks = [1, 1, 2, 2]
            while KT - sum(chunks) > 3:
                chunks.append(min(CH, KT - sum(chunks) - 3))
            chunks += [2, 1]
            kc = 0
            for ci, n in enumerate(chunks):
                last_chunk = ci == len(chunks) - 1
                t = xg_pool.tile([128, n * SLOT], bf16, tag="xg", name=f"xg{kc}")
                nc.sync.dma_start(t[:], xg[:, kc * SLOT : (kc + n) * SLOT])
                # b-major in the last chunk so each acc finishes (and its
                # copy-out can start) as early as possible.
                order = (
                    [(i, b) for b in range(n_b) for i in range(n)]
                    if last_chunk
                    else [(i, b) for i in range(n) for b in range(n_b)]
                )
                for i, b in order:
                    k = kc + i
                    nc.tensor.matmul(
                        accs[b][:],
                        t[:, i * SLOT + GOFF + b * 128 : i * SLOT + GOFF + (b + 1) * 128],
                        t[:, i * SLOT : i * SLOT + NK],
                        start=(k == 0),
                        stop=(k == KT - 1),
                    )
                kc += n
            # PSUM -> SBUF copies alternate vector/scalar so they drain
            # ~2x faster. Outputs leave in 2 grouped DMAs: a big one for
            # accs 0-5 as soon as they're copied, then a small final one
            # (accs 6-7) so the exposed DMA-completion latency at the very
            # end covers as few bytes as possible.
            o = out_pool.tile([128, n_b * NK], f32, tag="o", name="o")
            for b in range(n_b):
                if b % 2 == 0:
                    nc.vector.tensor_copy(o[:, b * NK : (b + 1) * NK], accs[b][:])
                else:
                    nc.scalar.copy(o[:, b * NK : (b + 1) * NK], accs[b][:])
                if b == 5:
                    nc.sync.dma_start(out[:, : 6 * NK], o[:, : 6 * NK])
            nc.sync.dma_start(out[:, 6 * NK :], o[:, 6 * NK :])
    nc.compile()
    return nc


def _get_nc():
    global _nc
    if _nc is None:
        _nc = _build()
    return _nc


def _pack_inputs(inputs, w, v):
    """Build per-core interleaved [128, KT*SLOT] bf16 streams."""
    import ml_dtypes

    bf16 = ml_dtypes.bfloat16
    FP = N_CORES * FPC
    XG = np.zeros((N_CORES, 128, KT, SLOT), dtype=bf16)
    # g part: [v | w] -> rows are features, cols are [312 v-cols, w, pad]
    Gv = XG[..., :NK].reshape(N_CORES, 128, KT, NK)
    G = np.zeros((FP, NK), dtype=bf16)
    G[:F, :NV] = v.reshape(F, NV).astype(bf16)
    G[:F, NL] = w[:, 0].astype(bf16)
    Gv[:] = G.reshape(N_CORES, KT, 128, NK).transpose(0, 2, 1, 3)
    # xt part: inputs^T
    XT = np.zeros((FP, B), dtype=bf16)
    XT[:F] = inputs.T.astype(bf16)
    XG[..., GOFF:] = XT.reshape(N_CORES, KT, 128, B).transpose(0, 2, 1, 3)
    return XG.reshape(N_CORES, 128, KT * SLOT)


def kernel(inputs, w0, w, v, _trace=False):
    global last_exec_time_ns
    from concourse.bass_utils import run_bass_kernel_spmd

    inputs = np.asarray(inputs, dtype=np.float32)
    w0 = np.asarray(w0, dtype=np.float32)
    w = np.asarray(w, dtype=np.float32)
    v = np.asarray(v, dtype=np.float32)

    XG = _pack_inputs(inputs, w, v)
    in_maps = [{"xg": XG[c]} for c in range(N_CORES)]
    nc = _get_nc()
    import os

    prev = os.environ.get("BASS_NEVER_TRACE")
    if not _trace:
        # Profiling needs an NTFF hook this container may not have; make
        # sure a stray BASS_TRACE env var can't pull us down that path.
        os.environ["BASS_NEVER_TRACE"] = "1"
    try:
        import time

        res = None
        for attempt in range(3):
            try:
                res = run_bass_kernel_spmd(
                    nc, in_maps, list(range(N_CORES)), trace=_trace
                )
                break
            except Exception:
                # Transient device wedges have been observed on shared
                # boxes; retry before giving up.
                if attempt == 2:
                    raise
                time.sleep(10)
    finally:
        if not _trace:
            if prev is None:
                os.environ.pop("BASS_NEVER_TRACE", None)
            else:
                os.environ["BASS_NEVER_TRACE"] = prev
    last_exec_time_ns = res.exec_time_ns

    total = np.zeros((B, NK), dtype=np.float64)
    for c in range(N_CORES):
        # device layout is [128, 8, NK] partition-major; batch row
        # r = j*128 + p lives at out[p, j*NK:(j+1)*NK]
        total += (
            res.results[c]["out"].reshape(128, B // 128, NK)
            .transpose(1, 0, 2)
            .reshape(B, NK)
        )

    field_f = total[:, :NV].reshape(B, FIELD, K)
    linear = total[:, NL] + np.float64(w0[0])
    s = field_f.sum(axis=1)                                     # [B, K]
    inter = 0.5 * ((s * s).sum(axis=-1) - (field_f * field_f).sum(axis=(1, 2)))
    return (linear + inter)[:, None].astype(np.float32)
